# revision 2
# baseline (speedup 1.0000x reference)
"""BERT attention (QKV proj + SDPA) sharded over 8 trn2 NeuronCores by head.

Problem: hidden_states [2, 2048, 1024], 16 heads x 64 dim, fp32.
Sharding: 2 heads per core (tensor-parallel on Q/K/V weight columns).

Per-core device kernel (matmul operands bf16, accumulation fp32):
  inputs:  xt  [1024, 4096]  X^T (host-pretransposed, bf16, same on all cores)
           wq/wk [1024, 128]  weight column slice for this core's 2 heads
           wv [1024, 128]
           bias [128, 2]      q/k bias slices packed (f32; V bias is added on
                              the host: softmax rows sum to 1 so bias_v just
                              adds to the output)
  output:  out [4096, 128] f32   context for the 2 heads (token-major)

Dataflow per batch:
  1. QT/KT [c=128, t] = W.T @ X.T (contraction over hidden), bias added on
     DVE during PSUM->SBUF copy.  V is projected directly in [token, dim]
     orientation (stationary = X^T hidden-chunk, moving = Wv) so V' [k, 65]
     per head needs no PE transpose; col 64 = ones (row sums).
  2. Scores TRANSPOSED: ST[k, q] f32 so softmax-exp output PT[k, q] serves
     as the STATIONARY operand of P@V (cost-model matmul time is the moving
     free size, so the 65-wide V' moving operand makes P@V ~2x cheaper than
     moving 512 q columns): ctx[q, d|sum] += PT[k, 128q].T @ V'[k, 65].
     exp has no max-subtraction (scores ~ N(0,1)); the 1/8 scale is folded
     into the ACT op.
  3. Normalize in-place (ctx rows are q tokens): per-partition reciprocal of
     the sums column; tensor_scalar multiply; DMA out with no transpose.

The attention loop is ACT(exp)-bound; projection matmul groups and P@V
tiles for earlier units are emitted as "fillers" inside the kt loop so the
PE does them under the exp shadow.  Unit 3 runs its two heads in sequential
phases so PV(u3, h0) hides under h1's exp shadow and only PV(u3, h1) is in
the tail.  PSUM: scores 2 tiles x 2 banks, proj 2x1 bank, ctx 2x1 bank.
"""

import numpy as np
import ml_dtypes

B, S, HID = 2, 2048, 1024
T = B * S
N_CORES = 8
P = 128
D = 64
HK = HID // P  # hidden-dim chunks

BF = ml_dtypes.bfloat16

_CACHED = {}


def _build():
    from collections import deque

    import concourse.bass as bass
    from concourse import bacc
    import concourse.tile as tile
    import concourse.mybir as mybir
    from concourse.bass import ts, ds
    from concourse.masks import make_identity

    bf16 = mybir.dt.bfloat16
    f32 = mybir.dt.float32
    Exp = mybir.ActivationFunctionType.Exp

    nc = bacc.Bacc(trn_type="TRN2", target_bir_lowering=False, debug=False)

    xt = nc.dram_tensor("xt", [HID, T], bf16, kind="ExternalInput").ap()
    wq = nc.dram_tensor("wq", [HID, P], bf16, kind="ExternalInput").ap()
    wk = nc.dram_tensor("wk", [HID, P], bf16, kind="ExternalInput").ap()
    wv = nc.dram_tensor("wv", [HID, P], bf16, kind="ExternalInput").ap()
    bias = nc.dram_tensor("bias", [P, 2], f32, kind="ExternalInput").ap()
    out = nc.dram_tensor("out", [T, P], f32, kind="ExternalOutput").ap()

    with tile.TileContext(nc) as tc:
        with (
            tc.tile_pool(name="const", bufs=1) as cpool,
            tc.tile_pool(name="xtp", bufs=1) as xtpool,
            tc.tile_pool(name="qkv", bufs=1) as qkvpool,
            tc.tile_pool(name="pt", bufs=1) as ptpool,
            tc.tile_pool(name="small", bufs=4) as smallpool,
            tc.tile_pool(name="ot", bufs=2) as otpool,
            tc.tile_pool(name="ps", bufs=2, space="PSUM") as psp,
        ):
            # X^T half-buffer: holds one batch's tokens; batch 1 reloads it
            # (all batch-0 projections are emitted before the reload DMAs).
            xt_sb = xtpool.tile([P, HK, S], bf16, tag="xt")
            xtp = xt.rearrange("(a p) t -> p a t", p=P)
            bias_sb = cpool.tile([P, 2], f32, tag="bias")
            b_sbs = [bias_sb[:, i : i + 1] for i in range(2)]
            wq_sb = cpool.tile([P, HK, P], bf16, tag="wq")
            wk_sb = cpool.tile([P, HK, P], bf16, tag="wk")
            wv_sb = cpool.tile([P, HK, P], bf16, tag="wv")
            w_sbs = [wq_sb, wk_sb]
            # DMA arrival order matched to first-consumption order so the PE
            # never idles mid-startup (transfers serialize on the DMA fabric)
            nc.sync.dma_start(xt_sb[:, :, 0:512], xtp[:, :, 0:512])
            nc.sync.dma_start(bias_sb, bias)
            nc.sync.dma_start(wq_sb, wq.rearrange("(a p) c -> p a c", p=P))
            nc.sync.dma_start(xt_sb[:, :, ts(1, 512)], xtp[:, :, ts(1, 512)])
            nc.sync.dma_start(wk_sb, wk.rearrange("(a p) c -> p a c", p=P))
            nc.sync.dma_start(wv_sb, wv.rearrange("(a p) c -> p a c", p=P))
            for quarter in range(2, 4):
                nc.sync.dma_start(
                    xt_sb[:, :, ts(quarter, 512)], xtp[:, :, ts(quarter, 512)]
                )

            ident_bf = cpool.tile([P, P], bf16, tag="identb")
            make_identity(nc, ident_bf)

            qt_sb = qkvpool.tile([P, T], bf16, tag="qt")
            kt_sb = qkvpool.tile([P, T], bf16, tag="kt")
            # V' per head: [k-part, global ktile, 65]; col 64 = ones (row sums)
            vp_sb = qkvpool.tile([P, 2, T // P, D + 1], bf16, tag="vp")
            nc.vector.memset(vp_sb[:, :, :, D : D + 1], 1.0)

            # PE warm-up while the first DMAs land: identity-only matmuls
            # ramp the HAM clock gate to full speed before real work. The
            # accumulated result is read once (into a V' slot that a later
            # v_proj overwrites) so DCE keeps the chain.
            wu = psp.tile([P, P], f32, tag="pj", bufs=2, name="wups")
            for i in range(24):
                nc.tensor.matmul(
                    wu, ident_bf, ident_bf, start=(i == 0), stop=(i == 23)
                )
            nc.vector.tensor_copy(vp_sb[:, 0, 0, 0:D], wu[:, 0:D])

            def qk_proj(t8, which):
                """Project 512 tokens (chunk t8) for q/k (which=0/1)."""
                w_sb, b_sb = w_sbs[which], b_sbs[which]
                dst = (qt_sb, kt_sb)[which]
                ps = psp.tile([P, 512], f32, tag="pj", bufs=2, name="projps")
                for a in range(HK):
                    nc.tensor.matmul(
                        ps,
                        w_sb[:, a, :],
                        xt_sb[:, a, ts(t8 % 4, 512)],
                        start=(a == 0),
                        stop=(a == HK - 1),
                    )
                nc.vector.tensor_scalar_add(dst[:, ts(t8, 512)], ps, b_sb)

            def v_proj_tile(t8, tt4):
                """Project one 128-token tile of V directly into V'[k, d]."""
                psv = psp.tile([P, P], f32, tag="pj", bufs=2, name="vps")
                for a in range(HK):
                    nc.tensor.matmul(
                        psv,
                        xt_sb[:, a, ds((t8 % 4) * 512 + tt4 * P, P)],
                        wv_sb[:, a, :],
                        start=(a == 0),
                        stop=(a == HK - 1),
                    )
                gk = (t8 // 4) * 16 + (t8 % 4) * 4 + tt4
                for h in range(2):
                    nc.vector.tensor_copy(
                        vp_sb[:, h, gk, 0:D], psv[:, h * D : (h + 1) * D]
                    )

            # PT ring: 2 heads x 32 slots x [128, 1024] bf16 (128 KB/partition)
            RING = 32
            pt_all = ptpool.tile([P, 2, RING, 1024], bf16, tag="pt")

            ot_cur = {}

            def pv_tile(unit, head, qt):
                """P@V + normalize for one (unit, head, 128-token q tile).

                ctx[q, d|sum] accumulates over the 16 buffered PT k-tiles
                with PT as the stationary operand (65 moving cols), then a
                per-partition reciprocal-normalize; no transpose needed."""
                b = unit // 2
                ctx = psp.tile([P, D + 1], f32, tag="ctx", bufs=2, name="ctx")
                for kt in range(16):
                    nc.tensor.matmul(
                        ctx,
                        pt_all[:, head, (unit * 16 + kt) % RING, ds(qt * P, P)],
                        vp_sb[:, head, b * 16 + kt, :],
                        start=(kt == 0),
                        stop=(kt == 15),
                    )
                rc = smallpool.tile([P, 1], f32, tag="rc")
                nc.vector.reciprocal(rc, ctx[:, D : D + 1])
                if qt == 0:
                    ot_cur[(unit, head)] = otpool.tile(
                        [P, 8, D], f32, tag="ot", name="ot"
                    )
                ot = ot_cur[(unit, head)]
                nc.vector.tensor_scalar_mul(ot[:, qt, :], ctx[:, 0:D], rc)
                if qt == 7:
                    qbase = b * S + (unit % 2) * 1024
                    hb = D * head
                    dst = out[ds(qbase, 1024), ds(hb, D)].rearrange(
                        "(tt p) d -> p tt d", p=P
                    )
                    nc.sync.dma_start(dst, ot)

            # Deferred-work queue: (cost, fn, deadline). Deadline (u, kt)
            # means the item MUST be emitted before (u, kt)'s scores/exp —
            # emission order is Tile's semantic order, so a late RAW
            # producer or a PT-ring WAR reader would read wrong data.
            # Items are popped by deadline (forced) or by cost pacing.
            # Unit 3 is head-phased: its kt key runs 0..31 (head*16 + kt).
            work_q = deque()

            def q_proj(t8, which, dl):
                work_q.append((1.7, lambda: qk_proj(t8, which), dl))

            def q_vproj(t8, dl):
                for tt4 in range(4):
                    work_q.append(
                        (0.5, lambda t=tt4: v_proj_tile(t8, t), dl)
                    )

            def q_pv(unit, dl, heads=(0, 1)):
                for head in heads:
                    for qt in range(8):
                        work_q.append(
                            (
                                0.5,
                                lambda h=head, q=qt: pv_tile(unit, h, q),
                                dl,
                            )
                        )

            NEVER = (9, 0)

            def push_unit_work(unit):
                if unit == 0:
                    # rest of batch 0 (essentials q0,q1,k0 already emitted)
                    q_proj(1, 1, (0, 4))  # k1
                    q_vproj(0, (1, 0))  # v0 (feeds pv(0) in unit 1)
                    q_vproj(1, (1, 0))
                    q_proj(2, 1, (0, 8))  # k2
                    q_vproj(2, (1, 0))
                    q_proj(2, 0, (1, 0))  # q2 (unit 1 scores)
                    q_proj(3, 1, (0, 12))  # k3
                    q_proj(3, 0, (1, 0))  # q3
                    q_vproj(3, (1, 0))
                elif unit == 1:
                    q_pv(0, (2, 0))  # PT slots reused by unit 2
                    q_proj(4, 1, (2, 0))  # k4
                    q_proj(4, 0, (2, 0))  # q4
                    q_proj(5, 0, (2, 0))  # q5
                    q_vproj(4, (3, 0))  # v4 feeds pv(2) in unit 3
                elif unit == 2:
                    q_proj(5, 1, (2, 4))  # k5
                    q_pv(1, (3, 0), heads=(0,))  # PT slots reused by unit 3
                    q_proj(6, 1, (2, 8))  # k6
                    q_pv(1, (3, 0), heads=(1,))
                    q_proj(7, 1, (2, 12))  # k7
                    q_proj(6, 0, (3, 0))  # q6
                    q_proj(7, 0, (3, 0))  # q7
                    q_vproj(5, (3, 0))
                elif unit == 3:
                    # rest of batch 1's V' (feeds pv(2); FIFO keeps them
                    # ahead), then pv(2) under phase-A's exp shadow
                    q_vproj(6, NEVER)
                    q_vproj(7, NEVER)
                    q_pv(2, NEVER)

            # ---- batch 0 essentials: just enough for unit 0's scores ----
            qk_proj(0, 0)  # q0
            qk_proj(1, 0)  # q1
            qk_proj(0, 1)  # k0

            def do_scores_exp(unit, head, kt, key):
                """One (head, kt): 2 score matmuls + 1 exp, plus queue pops."""
                b, qh = unit // 2, unit % 2
                base = b * S
                qbase = base + qh * 1024
                st = psp.tile([P, 1024], f32, tag="st", bufs=2, name="st")
                hb = D * head
                for j in range(2):
                    nc.tensor.matmul(
                        st[:, ts(j, 512)],
                        kt_sb[ds(hb, D), ds(base + kt * P, P)],
                        qt_sb[ds(hb, D), ds(qbase + j * 512, 512)],
                        start=True,
                        stop=True,
                    )
                nc.scalar.activation(
                    pt_all[:, head, (unit * 16 + kt) % RING, :],
                    st,
                    Exp,
                    scale=0.125,
                )

            credit = 2.0
            for unit in range(4):
                if unit == 1:
                    # drain every batch-0 consumer of xt_sb first: emission
                    # order is semantic order, so the reload must be emitted
                    # after all batch-0 projection reads
                    while work_q and work_q[0][2] <= (1, 0):
                        work_q.popleft()[1]()
                    # reload X^T with batch 1 tokens (WAR on batch-0 projs)
                    for quarter in range(4):
                        nc.sync.dma_start(
                            xt_sb[:, :, ts(quarter, 512)],
                            xtp[:, :, ds(S + quarter * 512, 512)],
                        )
                push_unit_work(unit)
                if unit < 3:
                    steps = [(kt, (0, 1)) for kt in range(16)]
                else:
                    # head-phased: h0's 16 kt, then h1's (kt key 0..31)
                    steps = [(kt, (kt // 16,)) for kt in range(32)]
                for kkey, heads in steps:
                    while work_q and work_q[0][2] <= (unit, kkey):
                        _, fn, _ = work_q.popleft()
                        fn()
                    for head in heads:
                        do_scores_exp(unit, head, kkey % 16, kkey)
                    if unit == 3 and kkey == 15:
                        # phase B begins: h0's PT tiles are final, its P@V
                        # runs under h1's exp shadow
                        q_pv(3, NEVER, heads=(0,))
                    # deferred work drained under the exp shadow, paced so
                    # the PE never runs far ahead of ACT
                    credit = min(credit + (1.4 if unit < 3 else 0.7), 8.0)
                    while work_q and work_q[0][0] <= credit:
                        cost, fn, _ = work_q.popleft()
                        credit -= cost
                        fn()
            while work_q:
                work_q.popleft()[1]()
            # tail: only h1's P@V depends on the last exps
            for qt in range(8):
                pv_tile(3, 1, qt)

    nc.compile()
    return nc


def get_nc():
    if "nc" not in _CACHED:
        _CACHED["nc"] = _build()
    return _CACHED["nc"]


def kernel(hidden_states, Wq, bq, Wk, bk, Wv, bv):
    from concourse.bass_utils import run_bass_kernel_spmd

    nc = get_nc()

    x2 = np.asarray(hidden_states, dtype=np.float32).reshape(T, HID)
    xt_b = np.ascontiguousarray(x2.T).astype(BF)
    bv_f = np.asarray(bv, np.float32)

    in_maps = []
    for c in range(N_CORES):
        sl = slice(P * c, P * (c + 1))
        in_maps.append(
            {
                "xt": xt_b,
                "wq": np.ascontiguousarray(np.asarray(Wq, np.float32)[:, sl]).astype(BF),
                "wk": np.ascontiguousarray(np.asarray(Wk, np.float32)[:, sl]).astype(BF),
                "wv": np.ascontiguousarray(np.asarray(Wv, np.float32)[:, sl]).astype(BF),
                "bias": np.ascontiguousarray(
                    np.stack(
                        [
                            np.asarray(bq, np.float32)[sl],
                            np.asarray(bk, np.float32)[sl],
                        ],
                        axis=1,
                    )
                ),
            }
        )

    res = run_bass_kernel_spmd(nc, in_maps, list(range(N_CORES)))

    full = np.empty((T, HID), dtype=np.float32)
    for c in range(N_CORES):
        # V bias: softmax rows sum to 1, so ctx(V + bv) = ctx(V) + bv exactly
        full[:, P * c : P * (c + 1)] = res.results[c]["out"] + bv_f[P * c : P * (c + 1)]
    return full.reshape(B, S, HID)


# revision 21
# speedup vs baseline: 1.1361x; 1.1361x over previous
"""BERT attention (QKV proj + SDPA) sharded over 8 trn2 NeuronCores by head.

Problem: hidden_states [2, 2048, 1024], 16 heads x 64 dim, fp32.
Sharding: 2 heads per core (tensor-parallel on Q/K/V weight columns).

Per-core device kernel (matmul operands bf16, accumulation fp32):
  inputs:  xt  [1024, 4096]  X^T (host-pretransposed, bf16, same on all cores)
           wq/wk [1024, 128]  weight column slice for this core's 2 heads
           wv [1024, 128]
           bias [128, 2]      q/k bias slices packed (f32; V bias is added on
                              the host: softmax rows sum to 1 so bias_v just
                              adds to the output)
  output:  out [4096, 128] f32   context for the 2 heads (token-major)

Dataflow per batch:
  1. QT/KT [c=128, t] = W.T @ X.T (contraction over hidden), bias added on
     DVE during PSUM->SBUF copy.  V is projected directly in [token, dim]
     orientation (stationary = X^T hidden-chunk, moving = Wv) so V' [k, 65]
     per head needs no PE transpose; col 64 = ones (row sums).
  2. Scores TRANSPOSED: ST[k, q] f32 so softmax-exp output PT[k, q] serves
     as the STATIONARY operand of P@V (cost-model matmul time is the moving
     free size, so the 65-wide V' moving operand makes P@V ~2x cheaper than
     moving 512 q columns): ctx[q, d|sum] += PT[k, 128q].T @ V'[k, 65].
     exp has no max-subtraction (scores ~ N(0,1)); the 1/8 scale is folded
     into the ACT op.
  3. Normalize in-place (ctx rows are q tokens): per-partition reciprocal of
     the sums column; tensor_scalar multiply; DMA out with no transpose.

The attention loop is ACT(exp)-bound; projection matmul groups and P@V
tiles for earlier units are emitted as "fillers" inside the kt loop so the
PE does them under the exp shadow.  Unit 3 runs its two heads in sequential
phases so PV(u3, h0) hides under h1's exp shadow and only PV(u3, h1) is in
the tail.  PSUM: scores 2 tiles x 2 banks, proj 2x1 bank, ctx 2x1 bank.
"""

import numpy as np
import ml_dtypes

B, S, HID = 2, 2048, 1024
T = B * S
N_CORES = 8
P = 128
D = 64
HK = HID // P  # hidden-dim chunks

BF = ml_dtypes.bfloat16

_CACHED = {}


def _build():
    from collections import deque

    import concourse.bass as bass
    from concourse import bacc
    import concourse.tile as tile
    import concourse.mybir as mybir
    from concourse.bass import ts, ds
    from concourse.masks import make_identity

    bf16 = mybir.dt.bfloat16
    f32 = mybir.dt.float32
    Exp = mybir.ActivationFunctionType.Exp

    nc = bacc.Bacc(trn_type="TRN2", target_bir_lowering=False, debug=False)

    xt = nc.dram_tensor("xt", [HID, T], bf16, kind="ExternalInput").ap()
    # weights host-packed [p, a*128+c] = W[a*128+p, c]: 2 KB contiguous per
    # partition so the DMA runs at full descriptor width
    wq = nc.dram_tensor("wq", [P, HID], bf16, kind="ExternalInput").ap()
    wk = nc.dram_tensor("wk", [P, HID], bf16, kind="ExternalInput").ap()
    wv = nc.dram_tensor("wv", [P, HID], bf16, kind="ExternalInput").ap()
    bias = nc.dram_tensor("bias", [P, 2], f32, kind="ExternalInput").ap()
    out = nc.dram_tensor("out", [T, P], f32, kind="ExternalOutput").ap()

    with tile.TileContext(nc) as tc:
        with (
            tc.tile_pool(name="const", bufs=1) as cpool,
            tc.tile_pool(name="xtp", bufs=1) as xtpool,
            tc.tile_pool(name="qkv", bufs=1) as qkvpool,
            tc.tile_pool(name="pt", bufs=1) as ptpool,
            tc.tile_pool(name="small", bufs=4) as smallpool,
            tc.tile_pool(name="ot", bufs=2) as otpool,
            tc.tile_pool(name="ps", bufs=2, space="PSUM") as psp,
        ):
            # X^T half-buffer: holds one batch's tokens; batch 1 reloads it
            # (all batch-0 projections are emitted before the reload DMAs).
            xt_sb = xtpool.tile([P, HK, S], bf16, tag="xt")
            xtp = xt.rearrange("(a p) t -> p a t", p=P)
            bias_sb = cpool.tile([P, 2], f32, tag="bias")
            b_sbs = [bias_sb[:, i : i + 1] for i in range(2)]
            wq_sb = cpool.tile([P, HK, P], bf16, tag="wq")
            wk_sb = cpool.tile([P, HK, P], bf16, tag="wk")
            wv_sb = cpool.tile([P, HK, P], bf16, tag="wv")
            w_sbs = [wq_sb, wk_sb]
            # DMA arrival order matched to first-consumption order; transfers
            # serialize on the (single-slot) DMA fabric in queue order, so the
            # first xt quarter is split in two 256-token pieces with the
            # (small) weight transfers slotted between them.
            nc.sync.dma_start(wq_sb, wq.rearrange("p (a c) -> p a c", a=HK))
            nc.sync.dma_start(xt_sb[:, :, 0:256], xtp[:, :, 0:256])
            nc.sync.dma_start(wk_sb, wk.rearrange("p (a c) -> p a c", a=HK))
            nc.sync.dma_start(bias_sb, bias)
            nc.sync.dma_start(xt_sb[:, :, 256:512], xtp[:, :, 256:512])
            nc.sync.dma_start(xt_sb[:, :, ts(1, 512)], xtp[:, :, ts(1, 512)])
            nc.sync.dma_start(wv_sb, wv.rearrange("p (a c) -> p a c", a=HK))
            for quarter in range(2, 4):
                nc.sync.dma_start(
                    xt_sb[:, :, ts(quarter, 512)], xtp[:, :, ts(quarter, 512)]
                )

            ident_bf = cpool.tile([P, P], bf16, tag="identb")
            make_identity(nc, ident_bf)

            qt_sb = qkvpool.tile([P, T], bf16, tag="qt")
            kt_sb = qkvpool.tile([P, T], bf16, tag="kt")
            # V' per head: [k-part, global ktile, 65]; col 64 = ones (row sums)
            vp_sb = qkvpool.tile([P, 2, T // P, D + 1], bf16, tag="vp")
            nc.vector.memset(vp_sb[:, :, :, D : D + 1], 1.0)

            # PE warm-up while the first DMAs land: identity-only matmuls
            # ramp the HAM clock gate to full speed before real work. The
            # accumulated result is read once (into a V' slot that a later
            # v_proj overwrites) so DCE keeps the chain.
            NWU = 38
            wu = psp.tile([P, P], f32, tag="pj", bufs=2, name="wups")
            for i in range(NWU):
                nc.tensor.matmul(
                    wu, ident_bf, ident_bf, start=(i == 0), stop=(i == NWU - 1)
                )
            nc.vector.tensor_copy(vp_sb[:, 0, 0, 0:D], wu[:, 0:D])

            def qk_proj(t8, which, half=None):
                """Project 512 tokens (chunk t8) for q/k (which=0/1).

                half=0/1 projects only 256 tokens (startup granularity)."""
                w_sb, b_sb = w_sbs[which], b_sbs[which]
                dst = (qt_sb, kt_sb)[which]
                t0, w = (0, 512) if half is None else (half * 256, 256)
                ps = psp.tile([P, 512], f32, tag="pj", bufs=2, name="projps")
                ps = ps[:, 0:w]
                for a in range(HK):
                    nc.tensor.matmul(
                        ps,
                        w_sb[:, a, :],
                        xt_sb[:, a, ds((t8 % 4) * 512 + t0, w)],
                        start=(a == 0),
                        stop=(a == HK - 1),
                    )
                # the bias-add gates later score groups: let it beat the
                # deferred fillers on DVE
                with tc.high_priority():
                    nc.vector.tensor_scalar_add(
                        dst[:, ds(t8 * 512 + t0, w)], ps, b_sb
                    )

            def v_proj_tile(t8, tt4):
                """Project one 128-token tile of V directly into V'[k, d]."""
                psv = psp.tile([P, P], f32, tag="pj", bufs=2, name="vps")
                for a in range(HK):
                    nc.tensor.matmul(
                        psv,
                        xt_sb[:, a, ds((t8 % 4) * 512 + tt4 * P, P)],
                        wv_sb[:, a, :],
                        start=(a == 0),
                        stop=(a == HK - 1),
                    )
                gk = (t8 // 4) * 16 + (t8 % 4) * 4 + tt4
                for h in range(2):
                    nc.vector.tensor_copy(
                        vp_sb[:, h, gk, 0:D], psv[:, h * D : (h + 1) * D]
                    )

            # PT ring: 2 heads x 32 slots x [128, 1024] bf16 (128 KB/partition)
            RING = 32
            pt_all = ptpool.tile([P, 2, RING, 1024], bf16, tag="pt")

            ot_cur = {}

            def pv_tile(unit, head, qt):
                """P@V + normalize for one (unit, head, 128-token q tile).

                ctx[q, d|sum] accumulates over the 16 buffered PT k-tiles
                with PT as the stationary operand (65 moving cols), then a
                per-partition reciprocal-normalize; no transpose needed."""
                b = unit // 2
                ctx = psp.tile([P, D + 1], f32, tag="ctx", bufs=2, name="ctx")
                for kt in range(16):
                    nc.tensor.matmul(
                        ctx,
                        pt_all[:, head, (unit * 16 + kt) % RING, ds(qt * P, P)],
                        vp_sb[:, head, b * 16 + kt, :],
                        start=(kt == 0),
                        stop=(kt == 15),
                    )
                rc = smallpool.tile([P, 1], f32, tag="rc")
                nc.vector.reciprocal(rc, ctx[:, D : D + 1])
                if qt == 0:
                    ot_cur[(unit, head)] = otpool.tile(
                        [P, 8, D], f32, tag="ot", name="ot"
                    )
                ot = ot_cur[(unit, head)]
                nc.vector.tensor_scalar_mul(ot[:, qt, :], ctx[:, 0:D], rc)
                # chunked DMAs so earlier chunks overlap later P@V; the very
                # last (unit, head) uses the finest chunks to shrink the tail
                nq = 2 if (unit, head) == (3, 1) else 4
                if (qt + 1) % nq == 0:
                    q0 = qt + 1 - nq
                    qbase = b * S + (unit % 2) * 1024 + q0 * 128
                    hb = D * head
                    dst = out[ds(qbase, nq * P), ds(hb, D)].rearrange(
                        "(tt p) d -> p tt d", p=P
                    )
                    nc.sync.dma_start(dst, ot[:, q0 : qt + 1, :])

            # Deferred-work queue: (cost, fn, deadline). Deadline (u, kt)
            # means the item MUST be emitted before (u, kt)'s scores/exp —
            # emission order is Tile's semantic order, so a late RAW
            # producer or a PT-ring WAR reader would read wrong data.
            # Items are popped by deadline (forced) or by cost pacing.
            # Unit 3 is head-phased: its kt key runs 0..31 (head*16 + kt).
            work_q = deque()

            def q_proj(t8, which, dl):
                work_q.append((1.7, lambda: qk_proj(t8, which), dl))

            def q_vproj(t8, dl):
                for tt4 in range(4):
                    work_q.append(
                        (0.5, lambda t=tt4: v_proj_tile(t8, t), dl)
                    )

            def q_pv(unit, dl, heads=(0, 1)):
                for head in heads:
                    for qt in range(8):
                        work_q.append(
                            (
                                0.5,
                                lambda h=head, q=qt: pv_tile(unit, h, q),
                                dl,
                            )
                        )

            NEVER = (9, 0)

            def push_unit_work(unit):
                # Projections first (they gate later score groups and so the
                # ACT-critical chain); the scheduler runs everything by
                # readiness with emission order as the tiebreak, so pv/v
                # fillers naturally yield to them.
                if unit == 0:
                    # rest of batch 0 (essentials q0,q1,k0 already emitted)
                    q_proj(1, 1, (0, 4))  # k1
                    q_proj(2, 1, (0, 8))  # k2
                    q_proj(3, 1, (0, 12))  # k3
                    q_proj(2, 0, (1, 0))  # q2 (unit 1 scores)
                    q_proj(3, 0, (1, 0))  # q3
                    q_vproj(0, (1, 0))  # v0..v3 feed pv(0) in unit 1
                    q_vproj(1, (1, 0))
                    q_vproj(2, (1, 0))
                    q_vproj(3, (1, 0))
                elif unit == 1:
                    q_proj(4, 1, (2, 0))  # k4
                    q_proj(4, 0, (2, 0))  # q4
                    q_proj(5, 0, (2, 0))  # q5
                    q_pv(0, (2, 0))  # PT slots reused by unit 2
                    q_vproj(4, (3, 0))  # v4 feeds pv(2) in unit 3
                elif unit == 2:
                    q_proj(5, 1, (2, 4))  # k5
                    q_proj(6, 1, (2, 8))  # k6
                    q_proj(7, 1, (2, 12))  # k7
                    q_proj(6, 0, (3, 0))  # q6
                    q_proj(7, 0, (3, 0))  # q7
                    q_pv(1, (3, 0))  # PT slots reused by unit 3
                    q_vproj(5, (3, 0))
                elif unit == 3:
                    # rest of batch 1's V' (feeds pv(2); FIFO keeps them
                    # ahead), then pv(2) under phase-A's exp shadow
                    q_vproj(6, NEVER)
                    q_vproj(7, NEVER)
                    q_pv(2, NEVER)

            def do_scores_exp(unit, head, kt, key):
                """One (head, kt): 2 score matmuls + 1 exp, plus queue pops.

                High priority: the score->exp chain is the ACT critical path,
                so score matmuls must preempt deferred fillers on the PE the
                moment their st WAR clears."""
                b, qh = unit // 2, unit % 2
                base = b * S
                qbase = base + qh * 1024
                st = psp.tile([P, 1024], f32, tag="st", bufs=2, name="st")
                hb = D * head
                with tc.high_priority():
                    for j in range(2):
                        nc.tensor.matmul(
                            st[:, ts(j, 512)],
                            kt_sb[ds(hb, D), ds(base + kt * P, P)],
                            qt_sb[ds(hb, D), ds(qbase + j * 512, 512)],
                            start=True,
                            stop=True,
                        )
                    nc.scalar.activation(
                        pt_all[:, head, (unit * 16 + kt) % RING, :],
                        st,
                        Exp,
                        scale=0.125,
                    )

            def scores_exp_part_u3h1(kt, j):
                """One 512-wide j-half of unit 3 / head 1's scores+exp."""
                hb = D
                slot = (48 + kt) % RING
                stj = psp.tile([P, 512], f32, tag="st", bufs=2, name="stj3")
                with tc.high_priority():
                    nc.tensor.matmul(
                        stj,
                        kt_sb[ds(hb, D), ds(S + kt * P, P)],
                        qt_sb[ds(hb, D), ds(S + 1024 + j * 512, 512)],
                        start=True,
                        stop=True,
                    )
                    nc.scalar.activation(
                        pt_all[:, 1, slot, ds(j * 512, 512)],
                        stj,
                        Exp,
                        scale=0.125,
                    )

            def scores_exp_part(head, kt, c0, w):
                """Unit-0 startup: one w-wide score matmul + exp so ACT can
                start before the full q half (and later k tiles) arrive."""
                hb = D * head
                stj = psp.tile([P, 512], f32, tag="st", bufs=2, name="stj")
                with tc.high_priority():
                    nc.tensor.matmul(
                        stj[:, 0:w],
                        kt_sb[ds(hb, D), ds(kt * P, P)],
                        qt_sb[ds(hb, D), ds(c0, w)],
                        start=True,
                        stop=True,
                    )
                    nc.scalar.activation(
                        pt_all[:, head, kt, ds(c0, w)],
                        stj[:, 0:w],
                        Exp,
                        scale=0.125,
                    )

            # ---- batch 0 startup: 256-token projection granularity and
            # split scores/exps for kt 0..3 so the first exp runs as soon
            # as the first 256 tokens + wq/wk have landed ----
            qk_proj(0, 0, half=0)  # q0a
            qk_proj(0, 1, half=0)  # k0a -> ktiles 0,1
            for kt in (0, 1):
                for head in range(2):
                    scores_exp_part(head, kt, 0, 256)
            qk_proj(0, 0, half=1)  # q0b
            for kt in (0, 1):
                for head in range(2):
                    scores_exp_part(head, kt, 256, 256)
            qk_proj(0, 1, half=1)  # k0b -> ktiles 2,3
            for kt in (2, 3):
                for head in range(2):
                    scores_exp_part(head, kt, 0, 512)
            qk_proj(1, 0)  # q1
            for kt in range(4):
                for head in range(2):
                    scores_exp_part(head, kt, 512, 512)

            credit = 2.0
            for unit in range(4):
                if unit == 1:
                    # drain every batch-0 consumer of xt_sb first: emission
                    # order is semantic order, so the reload must be emitted
                    # after all batch-0 projection reads
                    while work_q and work_q[0][2] <= (1, 0):
                        work_q.popleft()[1]()
                    # reload X^T with batch 1 tokens (WAR on batch-0 projs)
                    for quarter in range(4):
                        nc.sync.dma_start(
                            xt_sb[:, :, ts(quarter, 512)],
                            xtp[:, :, ds(S + quarter * 512, 512)],
                        )
                push_unit_work(unit)
                if unit == 0:
                    steps = [(kt, (0, 1)) for kt in range(4, 16)]
                elif unit < 3:
                    steps = [(kt, (0, 1)) for kt in range(16)]
                else:
                    # head-phased: h0's 16 kt, then h1's (kt key 0..31)
                    steps = [(kt, (kt // 16,)) for kt in range(32)]
                for kkey, heads in steps:
                    while work_q and work_q[0][2] <= (unit, kkey):
                        _, fn, _ = work_q.popleft()
                        fn()
                    if unit == 3 and kkey == 31:
                        # j-split the very last exp: the tail P@V for q tiles
                        # 0-3 only needs the j0 half, so it starts one
                        # half-exp earlier
                        for j in range(2):
                            scores_exp_part_u3h1(15, j)
                        continue
                    for head in heads:
                        do_scores_exp(unit, head, kkey % 16, kkey)
                    if unit == 3 and kkey == 15:
                        # phase B begins: h0's PT tiles are final, its P@V
                        # runs under h1's exp shadow
                        q_pv(3, NEVER, heads=(0,))
                    # deferred work drained under the exp shadow, paced so
                    # the PE never runs far ahead of ACT
                    credit = min(credit + (1.4 if unit < 3 else 0.7), 8.0)
                    while work_q and work_q[0][0] <= credit:
                        cost, fn, _ = work_q.popleft()
                        credit -= cost
                        fn()
            while work_q:
                work_q.popleft()[1]()
            # tail: only h1's P@V depends on the last exps
            for qt in range(8):
                pv_tile(3, 1, qt)

    nc.compile()
    return nc


def get_nc():
    if "nc" not in _CACHED:
        _CACHED["nc"] = _build()
    return _CACHED["nc"]


def kernel(hidden_states, Wq, bq, Wk, bk, Wv, bv):
    from concourse.bass_utils import run_bass_kernel_spmd

    nc = get_nc()

    x2 = np.asarray(hidden_states, dtype=np.float32).reshape(T, HID)
    xt_b = np.ascontiguousarray(x2.T).astype(BF)
    bv_f = np.asarray(bv, np.float32)

    def pack_w(W, sl):
        # [p, a*128+c] = W[a*128+p, c]: 2 KB contiguous rows for fast DMA
        w = np.asarray(W, np.float32)[:, sl].reshape(HK, P, P)
        return np.ascontiguousarray(w.transpose(1, 0, 2).reshape(P, HID)).astype(BF)

    in_maps = []
    for c in range(N_CORES):
        sl = slice(P * c, P * (c + 1))
        in_maps.append(
            {
                "xt": xt_b,
                "wq": pack_w(Wq, sl),
                "wk": pack_w(Wk, sl),
                "wv": pack_w(Wv, sl),
                "bias": np.ascontiguousarray(
                    np.stack(
                        [
                            np.asarray(bq, np.float32)[sl],
                            np.asarray(bk, np.float32)[sl],
                        ],
                        axis=1,
                    )
                ),
            }
        )

    res = run_bass_kernel_spmd(nc, in_maps, list(range(N_CORES)))

    full = np.empty((T, HID), dtype=np.float32)
    for c in range(N_CORES):
        # V bias: softmax rows sum to 1, so ctx(V + bv) = ctx(V) + bv exactly
        full[:, P * c : P * (c + 1)] = res.results[c]["out"] + bv_f[P * c : P * (c + 1)]
    return full.reshape(B, S, HID)


# revision 25
# speedup vs baseline: 1.1468x; 1.0095x over previous
"""BERT attention (QKV proj + SDPA) sharded over 8 trn2 NeuronCores by head.

Problem: hidden_states [2, 2048, 1024], 16 heads x 64 dim, fp32.
Sharding: 2 heads per core (tensor-parallel on Q/K/V weight columns).

Per-core device kernel (matmul operands bf16, accumulation fp32):
  inputs:  xt  [1024, 4096]  X^T (host-pretransposed, bf16, same on all cores)
           wq/wk [1024, 128]  weight column slice for this core's 2 heads
           wv [1024, 128]
           bias [128, 2]      q/k bias slices packed (f32; V bias is added on
                              the host: softmax rows sum to 1 so bias_v just
                              adds to the output)
  output:  out [4096, 128] f32   context for the 2 heads (token-major)

Dataflow per batch:
  1. QT/KT [c=128, t] = W.T @ X.T (contraction over hidden), bias added on
     DVE during PSUM->SBUF copy.  V is projected directly in [token, dim]
     orientation (stationary = X^T hidden-chunk, moving = Wv) so V' [k, 65]
     per head needs no PE transpose; col 64 = ones (row sums).
  2. Scores TRANSPOSED: ST[k, q] f32 so softmax-exp output PT[k, q] serves
     as the STATIONARY operand of P@V (cost-model matmul time is the moving
     free size, so the 65-wide V' moving operand makes P@V ~2x cheaper than
     moving 512 q columns): ctx[q, d|sum] += PT[k, 128q].T @ V'[k, 65].
     exp has no max-subtraction (scores ~ N(0,1)); the 1/8 scale is folded
     into the ACT op.
  3. Normalize in-place (ctx rows are q tokens): per-partition reciprocal of
     the sums column; tensor_scalar multiply; DMA out with no transpose.

The attention loop is ACT(exp)-bound; projection matmul groups and P@V
tiles for earlier units are emitted as "fillers" inside the kt loop so the
PE does them under the exp shadow.  Unit 3 runs its two heads in sequential
phases so PV(u3, h0) hides under h1's exp shadow and only PV(u3, h1) is in
the tail.  PSUM: scores 2 tiles x 2 banks, proj 2x1 bank, ctx 2x1 bank.
"""

import numpy as np
import ml_dtypes

B, S, HID = 2, 2048, 1024
T = B * S
N_CORES = 8
P = 128
D = 64
HK = HID // P  # hidden-dim chunks

BF = ml_dtypes.bfloat16

_CACHED = {}


def _build():
    from collections import deque

    import concourse.bass as bass
    from concourse import bacc
    import concourse.tile as tile
    import concourse.mybir as mybir
    from concourse.bass import ts, ds
    from concourse.masks import make_identity

    bf16 = mybir.dt.bfloat16
    f32 = mybir.dt.float32
    Exp = mybir.ActivationFunctionType.Exp

    nc = bacc.Bacc(trn_type="TRN2", target_bir_lowering=False, debug=False)

    xt = nc.dram_tensor("xt", [HID, T], bf16, kind="ExternalInput").ap()
    # weights host-packed [p, a*128+c] = W[a*128+p, c]: 2 KB contiguous per
    # partition so the DMA runs at full descriptor width
    wq = nc.dram_tensor("wq", [P, HID], bf16, kind="ExternalInput").ap()
    wk = nc.dram_tensor("wk", [P, HID], bf16, kind="ExternalInput").ap()
    wv = nc.dram_tensor("wv", [P, HID], bf16, kind="ExternalInput").ap()
    bias = nc.dram_tensor("bias", [P, 2], f32, kind="ExternalInput").ap()
    out = nc.dram_tensor("out", [T, P], f32, kind="ExternalOutput").ap()

    with tile.TileContext(nc) as tc:
        with (
            tc.tile_pool(name="const", bufs=1) as cpool,
            tc.tile_pool(name="xtp", bufs=1) as xtpool,
            tc.tile_pool(name="qkv", bufs=1) as qkvpool,
            tc.tile_pool(name="pt", bufs=1) as ptpool,
            tc.tile_pool(name="small", bufs=4) as smallpool,
            tc.tile_pool(name="ot", bufs=2) as otpool,
            tc.tile_pool(name="ps", bufs=2, space="PSUM") as psp,
        ):
            # X^T half-buffer: holds one batch's tokens; batch 1 reloads it
            # (all batch-0 projections are emitted before the reload DMAs).
            xt_sb = xtpool.tile([P, HK, S], bf16, tag="xt")
            xtp = xt.rearrange("(a p) t -> p a t", p=P)
            bias_sb = cpool.tile([P, 2], f32, tag="bias")
            b_sbs = [bias_sb[:, i : i + 1] for i in range(2)]
            wq_sb = cpool.tile([P, HK, P], bf16, tag="wq")
            wk_sb = cpool.tile([P, HK, P], bf16, tag="wk")
            wv_sb = cpool.tile([P, HK, P], bf16, tag="wv")
            w_sbs = [wq_sb, wk_sb]
            # DMA arrival order matched to first-consumption order; transfers
            # serialize on the (single-slot) DMA fabric in queue order, so the
            # first xt quarter is split in two 256-token pieces with the
            # (small) weight transfers slotted between them.
            nc.sync.dma_start(wq_sb, wq.rearrange("p (a c) -> p a c", a=HK))
            nc.sync.dma_start(xt_sb[:, :, 0:256], xtp[:, :, 0:256])
            nc.sync.dma_start(wk_sb, wk.rearrange("p (a c) -> p a c", a=HK))
            nc.sync.dma_start(bias_sb, bias)
            nc.sync.dma_start(xt_sb[:, :, 256:512], xtp[:, :, 256:512])
            nc.sync.dma_start(xt_sb[:, :, ts(1, 512)], xtp[:, :, ts(1, 512)])
            nc.sync.dma_start(wv_sb, wv.rearrange("p (a c) -> p a c", a=HK))
            for quarter in range(2, 4):
                nc.sync.dma_start(
                    xt_sb[:, :, ts(quarter, 512)], xtp[:, :, ts(quarter, 512)]
                )

            ident_bf = cpool.tile([P, P], bf16, tag="identb")
            make_identity(nc, ident_bf)

            qt_sb = qkvpool.tile([P, T], bf16, tag="qt")
            kt_sb = qkvpool.tile([P, T], bf16, tag="kt")
            # V' per head: [k-part, global ktile, 65]; col 64 = ones (row sums)
            vp_sb = qkvpool.tile([P, 2, T // P, D + 1], bf16, tag="vp")
            nc.vector.memset(vp_sb[:, :, :, D : D + 1], 1.0)

            # PE warm-up while the first DMAs land: identity-only matmuls
            # ramp the HAM clock gate to full speed before real work. The
            # accumulated result is read once (into a V' slot that a later
            # v_proj overwrites) so DCE keeps the chain.
            NWU = 38
            wu = psp.tile([P, P], f32, tag="pj", bufs=2, name="wups")
            for i in range(NWU):
                nc.tensor.matmul(
                    wu, ident_bf, ident_bf, start=(i == 0), stop=(i == NWU - 1)
                )
            nc.vector.tensor_copy(vp_sb[:, 0, 0, 0:D], wu[:, 0:D])

            def qk_proj(t8, which, half=None):
                """Project 512 tokens (chunk t8) for q/k (which=0/1).

                half=0/1 projects only 256 tokens (startup granularity)."""
                w_sb, b_sb = w_sbs[which], b_sbs[which]
                dst = (qt_sb, kt_sb)[which]
                t0, w = (0, 512) if half is None else (half * 256, 256)
                ps = psp.tile([P, 512], f32, tag="pj", bufs=2, name="projps")
                ps = ps[:, 0:w]
                for a in range(HK):
                    nc.tensor.matmul(
                        ps,
                        w_sb[:, a, :],
                        xt_sb[:, a, ds((t8 % 4) * 512 + t0, w)],
                        start=(a == 0),
                        stop=(a == HK - 1),
                    )
                # the bias-add gates later score groups: let it beat the
                # deferred fillers on DVE
                with tc.high_priority():
                    nc.vector.tensor_scalar_add(
                        dst[:, ds(t8 * 512 + t0, w)], ps, b_sb
                    )

            def v_proj_tile(t8, tt4):
                """Project one 128-token tile of V directly into V'[k, d]."""
                psv = psp.tile([P, P], f32, tag="pj", bufs=2, name="vps")
                for a in range(HK):
                    nc.tensor.matmul(
                        psv,
                        xt_sb[:, a, ds((t8 % 4) * 512 + tt4 * P, P)],
                        wv_sb[:, a, :],
                        start=(a == 0),
                        stop=(a == HK - 1),
                    )
                gk = (t8 // 4) * 16 + (t8 % 4) * 4 + tt4
                for h in range(2):
                    nc.vector.tensor_copy(
                        vp_sb[:, h, gk, 0:D], psv[:, h * D : (h + 1) * D]
                    )

            # PT ring: 2 heads x 32 slots x [128, 1024] bf16 (128 KB/partition)
            RING = 32
            pt_all = ptpool.tile([P, 2, RING, 1024], bf16, tag="pt")

            ot_cur = {}

            def pv_norm(unit, head, qt, ctx):
                """Reciprocal-normalize one finished ctx tile + chunked DMA."""
                b = unit // 2
                rc = smallpool.tile([P, 1], f32, tag="rc")
                nc.vector.reciprocal(rc, ctx[:, D : D + 1])
                if qt == 0:
                    ot_cur[(unit, head)] = otpool.tile(
                        [P, 8, D], f32, tag="ot", name="ot"
                    )
                ot = ot_cur[(unit, head)]
                nc.vector.tensor_scalar_mul(ot[:, qt, :], ctx[:, 0:D], rc)
                # chunked DMAs so earlier chunks overlap later P@V; the very
                # last (unit, head) uses finer chunks to shrink the tail
                chunks = {3: 4, 5: 2, 7: 2} if (unit, head) == (3, 1) else {3: 4, 7: 4}
                if qt in chunks:
                    nq = chunks[qt]
                    q0 = qt + 1 - nq
                    qbase = b * S + (unit % 2) * 1024 + q0 * 128
                    hb = D * head
                    dst = out[ds(qbase, nq * P), ds(hb, D)].rearrange(
                        "(tt p) d -> p tt d", p=P
                    )
                    nc.sync.dma_start(dst, ot[:, q0 : qt + 1, :])

            def pv_tile(unit, head, qt, tag="ctx"):
                """P@V + normalize for one (unit, head, 128-token q tile).

                ctx[q, d|sum] accumulates over the 16 buffered PT k-tiles
                with PT as the stationary operand (65 moving cols), then a
                per-partition reciprocal-normalize; no transpose needed."""
                b = unit // 2
                ctx = psp.tile([P, D + 1], f32, tag=tag, bufs=2, name="ctx")
                for kt in range(16):
                    nc.tensor.matmul(
                        ctx,
                        pt_all[:, head, (unit * 16 + kt) % RING, ds(qt * P, P)],
                        vp_sb[:, head, b * 16 + kt, :],
                        start=(kt == 0),
                        stop=(kt == 15),
                    )
                pv_norm(unit, head, qt, ctx)

            # Tail pre-accumulation for the last (unit 3, head 1) P@V: q
            # tiles 0-3 accumulate kt 0..14 under the last exps' shadow on
            # the 4 free PSUM banks (ctx + the by-then-idle pj tag), leaving
            # only the kt15 matmul + normalize after the final exp.
            pv31_ctx = {}

            def pv31_open(qt, tag):
                ctx = psp.tile([P, D + 1], f32, tag=tag, bufs=2, name="ctx31")
                for kt in range(15):
                    nc.tensor.matmul(
                        ctx,
                        pt_all[:, 1, (48 + kt) % RING, ds(qt * P, P)],
                        vp_sb[:, 1, 16 + kt, :],
                        start=(kt == 0),
                        stop=False,
                    )
                pv31_ctx[qt] = ctx

            def pv31_close(qt):
                ctx = pv31_ctx[qt]
                nc.tensor.matmul(
                    ctx,
                    pt_all[:, 1, 31, ds(qt * P, P)],
                    vp_sb[:, 1, 31, :],
                    start=False,
                    stop=True,
                )
                pv_norm(3, 1, qt, ctx)

            # Deferred-work queue: (cost, fn, deadline). Deadline (u, kt)
            # means the item MUST be emitted before (u, kt)'s scores/exp —
            # emission order is Tile's semantic order, so a late RAW
            # producer or a PT-ring WAR reader would read wrong data.
            # Items are popped by deadline (forced) or by cost pacing.
            # Unit 3 is head-phased: its kt key runs 0..31 (head*16 + kt).
            work_q = deque()

            def q_proj(t8, which, dl):
                work_q.append((1.7, lambda: qk_proj(t8, which), dl))

            def q_vproj(t8, dl):
                for tt4 in range(4):
                    work_q.append(
                        (0.5, lambda t=tt4: v_proj_tile(t8, t), dl)
                    )

            def q_pv(unit, dl, heads=(0, 1)):
                for head in heads:
                    for qt in range(8):
                        work_q.append(
                            (
                                0.5,
                                lambda h=head, q=qt: pv_tile(unit, h, q),
                                dl,
                            )
                        )

            NEVER = (9, 0)

            def push_unit_work(unit):
                # Projections first (they gate later score groups and so the
                # ACT-critical chain); the scheduler runs everything by
                # readiness with emission order as the tiebreak, so pv/v
                # fillers naturally yield to them.
                if unit == 0:
                    # rest of batch 0 (essentials q0,q1,k0 already emitted)
                    q_proj(1, 1, (0, 4))  # k1
                    q_proj(2, 1, (0, 8))  # k2
                    q_proj(3, 1, (0, 12))  # k3
                    q_proj(2, 0, (1, 0))  # q2 (unit 1 scores)
                    q_proj(3, 0, (1, 0))  # q3
                    q_vproj(0, (1, 0))  # v0..v3 feed pv(0) in unit 1
                    q_vproj(1, (1, 0))
                    q_vproj(2, (1, 0))
                    q_vproj(3, (1, 0))
                elif unit == 1:
                    q_proj(4, 1, (2, 0))  # k4
                    q_proj(4, 0, (2, 0))  # q4
                    q_proj(5, 0, (2, 0))  # q5
                    q_pv(0, (2, 0))  # PT slots reused by unit 2
                    q_vproj(4, (3, 0))  # v4 feeds pv(2) in unit 3
                elif unit == 2:
                    q_proj(5, 1, (2, 4))  # k5
                    q_proj(6, 1, (2, 8))  # k6
                    q_proj(7, 1, (2, 12))  # k7
                    q_proj(6, 0, (3, 0))  # q6
                    q_proj(7, 0, (3, 0))  # q7
                    q_pv(1, (3, 0))  # PT slots reused by unit 3
                    q_vproj(5, (3, 0))
                elif unit == 3:
                    # rest of batch 1's V' (feeds pv(2); FIFO keeps them
                    # ahead), then pv(2) under phase-A's exp shadow
                    q_vproj(6, NEVER)
                    q_vproj(7, NEVER)
                    q_pv(2, NEVER)

            def do_scores_exp(unit, head, kt, key):
                """One (head, kt): 2 score matmuls + 1 exp, plus queue pops.

                High priority: the score->exp chain is the ACT critical path,
                so score matmuls must preempt deferred fillers on the PE the
                moment their st WAR clears."""
                b, qh = unit // 2, unit % 2
                base = b * S
                qbase = base + qh * 1024
                st = psp.tile([P, 1024], f32, tag="st", bufs=2, name="st")
                hb = D * head
                with tc.high_priority():
                    for j in range(2):
                        nc.tensor.matmul(
                            st[:, ts(j, 512)],
                            kt_sb[ds(hb, D), ds(base + kt * P, P)],
                            qt_sb[ds(hb, D), ds(qbase + j * 512, 512)],
                            start=True,
                            stop=True,
                        )
                    nc.scalar.activation(
                        pt_all[:, head, (unit * 16 + kt) % RING, :],
                        st,
                        Exp,
                        scale=0.125,
                    )

            def scores_exp_part_u3h1(kt, j):
                """One 512-wide j-half of unit 3 / head 1's scores+exp."""
                hb = D
                slot = (48 + kt) % RING
                stj = psp.tile([P, 512], f32, tag="st", bufs=2, name="stj3")
                with tc.high_priority():
                    nc.tensor.matmul(
                        stj,
                        kt_sb[ds(hb, D), ds(S + kt * P, P)],
                        qt_sb[ds(hb, D), ds(S + 1024 + j * 512, 512)],
                        start=True,
                        stop=True,
                    )
                    nc.scalar.activation(
                        pt_all[:, 1, slot, ds(j * 512, 512)],
                        stj,
                        Exp,
                        scale=0.125,
                    )

            def scores_exp_part(head, kt, c0, w):
                """Unit-0 startup: one w-wide score matmul + exp so ACT can
                start before the full q half (and later k tiles) arrive."""
                hb = D * head
                stj = psp.tile([P, 512], f32, tag="st", bufs=2, name="stj")
                with tc.high_priority():
                    nc.tensor.matmul(
                        stj[:, 0:w],
                        kt_sb[ds(hb, D), ds(kt * P, P)],
                        qt_sb[ds(hb, D), ds(c0, w)],
                        start=True,
                        stop=True,
                    )
                    nc.scalar.activation(
                        pt_all[:, head, kt, ds(c0, w)],
                        stj[:, 0:w],
                        Exp,
                        scale=0.125,
                    )

            # ---- batch 0 startup: 256-token projection granularity and
            # split scores/exps for kt 0..3 so the first exp runs as soon
            # as the first 256 tokens + wq/wk have landed ----
            qk_proj(0, 0, half=0)  # q0a
            qk_proj(0, 1, half=0)  # k0a -> ktiles 0,1
            for kt in (0, 1):
                for head in range(2):
                    scores_exp_part(head, kt, 0, 256)
            qk_proj(0, 0, half=1)  # q0b
            for kt in (0, 1):
                for head in range(2):
                    scores_exp_part(head, kt, 256, 256)
            qk_proj(0, 1, half=1)  # k0b -> ktiles 2,3
            for kt in (2, 3):
                for head in range(2):
                    scores_exp_part(head, kt, 0, 512)
            qk_proj(1, 0)  # q1
            for kt in range(4):
                for head in range(2):
                    scores_exp_part(head, kt, 512, 512)

            credit = 2.0
            for unit in range(4):
                if unit == 1:
                    # drain every batch-0 consumer of xt_sb first: emission
                    # order is semantic order, so the reload must be emitted
                    # after all batch-0 projection reads
                    while work_q and work_q[0][2] <= (1, 0):
                        work_q.popleft()[1]()
                    # reload X^T with batch 1 tokens (WAR on batch-0 projs)
                    for quarter in range(4):
                        nc.sync.dma_start(
                            xt_sb[:, :, ts(quarter, 512)],
                            xtp[:, :, ds(S + quarter * 512, 512)],
                        )
                push_unit_work(unit)
                if unit == 0:
                    steps = [(kt, (0, 1)) for kt in range(4, 16)]
                elif unit < 3:
                    steps = [(kt, (0, 1)) for kt in range(16)]
                else:
                    # head-phased: h0's 16 kt, then h1's (kt key 0..31)
                    steps = [(kt, (kt // 16,)) for kt in range(32)]
                for kkey, heads in steps:
                    while work_q and work_q[0][2] <= (unit, kkey):
                        _, fn, _ = work_q.popleft()
                        fn()
                    if unit == 3 and kkey == 31:
                        # j-split the very last exp: the tail P@V for q tiles
                        # 0-3 only needs the j0 half, so it closes right
                        # after it while the j1 half still runs
                        scores_exp_part_u3h1(15, 0)
                        for qt in range(4):
                            pv31_close(qt)
                        scores_exp_part_u3h1(15, 1)
                        continue
                    for head in heads:
                        do_scores_exp(unit, head, kkey % 16, kkey)
                    if unit == 3 and kkey == 30:
                        # kt 0..14 PT tiles are final: pre-accumulate the
                        # tail's first 4 q tiles under the last exps
                        for qt, tag in ((0, "ctx"), (1, "ctx"), (2, "pj"), (3, "pj")):
                            pv31_open(qt, tag)
                    if unit == 3 and kkey == 15:
                        # phase B begins: h0's PT tiles are final, its P@V
                        # runs under h1's exp shadow
                        q_pv(3, NEVER, heads=(0,))
                    # deferred work drained under the exp shadow, paced so
                    # the PE never runs far ahead of ACT
                    credit = min(credit + (1.4 if unit < 3 else 0.7), 8.0)
                    while work_q and work_q[0][0] <= credit:
                        cost, fn, _ = work_q.popleft()
                        credit -= cost
                        fn()
            while work_q:
                work_q.popleft()[1]()
            # tail: only h1's last 4 q tiles remain (0-3 closed in-loop).
            # qt4/5 take the st banks (free the moment the last exps read
            # them); qt6/7 take the slots qt0/qt2's norms release.
            for qt, tag in ((4, "st"), (5, "st"), (6, "ctx"), (7, "pj")):
                pv_tile(3, 1, qt, tag=tag)

    nc.compile()
    return nc


def get_nc():
    if "nc" not in _CACHED:
        _CACHED["nc"] = _build()
    return _CACHED["nc"]


def kernel(hidden_states, Wq, bq, Wk, bk, Wv, bv):
    from concourse.bass_utils import run_bass_kernel_spmd

    nc = get_nc()

    x2 = np.asarray(hidden_states, dtype=np.float32).reshape(T, HID)
    xt_b = np.ascontiguousarray(x2.T).astype(BF)
    bv_f = np.asarray(bv, np.float32)

    def pack_w(W, sl):
        # [p, a*128+c] = W[a*128+p, c]: 2 KB contiguous rows for fast DMA
        w = np.asarray(W, np.float32)[:, sl].reshape(HK, P, P)
        return np.ascontiguousarray(w.transpose(1, 0, 2).reshape(P, HID)).astype(BF)

    in_maps = []
    for c in range(N_CORES):
        sl = slice(P * c, P * (c + 1))
        in_maps.append(
            {
                "xt": xt_b,
                "wq": pack_w(Wq, sl),
                "wk": pack_w(Wk, sl),
                "wv": pack_w(Wv, sl),
                "bias": np.ascontiguousarray(
                    np.stack(
                        [
                            np.asarray(bq, np.float32)[sl],
                            np.asarray(bk, np.float32)[sl],
                        ],
                        axis=1,
                    )
                ),
            }
        )

    res = run_bass_kernel_spmd(nc, in_maps, list(range(N_CORES)))

    full = np.empty((T, HID), dtype=np.float32)
    for c in range(N_CORES):
        # V bias: softmax rows sum to 1, so ctx(V + bv) = ctx(V) + bv exactly
        full[:, P * c : P * (c + 1)] = res.results[c]["out"] + bv_f[P * c : P * (c + 1)]
    return full.reshape(B, S, HID)


# revision 29
# speedup vs baseline: 1.1556x; 1.0077x over previous
"""BERT attention (QKV proj + SDPA) sharded over 8 trn2 NeuronCores by head.

Problem: hidden_states [2, 2048, 1024], 16 heads x 64 dim, fp32.
Sharding: 2 heads per core (tensor-parallel on Q/K/V weight columns).

Per-core device kernel (matmul operands bf16, accumulation fp32):
  inputs:  xt  [1024, 4096]  X^T (host-pretransposed, bf16, same on all cores)
           wq/wk [1024, 128]  weight column slice for this core's 2 heads
           wv [1024, 128]
           bias [128, 2]      q/k bias slices packed (f32; V bias is added on
                              the host: softmax rows sum to 1 so bias_v just
                              adds to the output)
  output:  out [4096, 128] f32   context for the 2 heads (token-major)

Dataflow per batch:
  1. QT/KT [c=128, t] = W.T @ X.T (contraction over hidden), bias added on
     DVE during PSUM->SBUF copy.  V is projected directly in [token, dim]
     orientation (stationary = X^T hidden-chunk, moving = Wv) so V' [k, 65]
     per head needs no PE transpose; col 64 = ones (row sums).
  2. Scores TRANSPOSED: ST[k, q] f32 so softmax-exp output PT[k, q] serves
     as the STATIONARY operand of P@V (cost-model matmul time is the moving
     free size, so the 65-wide V' moving operand makes P@V ~2x cheaper than
     moving 512 q columns): ctx[q, d|sum] += PT[k, 128q].T @ V'[k, 65].
     exp has no max-subtraction (scores ~ N(0,1)); the 1/8 scale is folded
     into the ACT op.
  3. Normalize in-place (ctx rows are q tokens): per-partition reciprocal of
     the sums column; tensor_scalar multiply; DMA out with no transpose.

The attention loop is ACT(exp)-bound; projection matmul groups and P@V
tiles for earlier units are emitted as "fillers" inside the kt loop so the
PE does them under the exp shadow.  Unit 3 runs its two heads in sequential
phases so PV(u3, h0) hides under h1's exp shadow and only PV(u3, h1) is in
the tail.  PSUM: scores 2 tiles x 2 banks, proj 2x1 bank, ctx 2x1 bank.
"""

import numpy as np
import ml_dtypes

B, S, HID = 2, 2048, 1024
T = B * S
N_CORES = 8
P = 128
D = 64
HK = HID // P  # hidden-dim chunks

BF = ml_dtypes.bfloat16

_CACHED = {}


def _build():
    from collections import deque

    import concourse.bass as bass
    from concourse import bacc
    import concourse.tile as tile
    import concourse.mybir as mybir
    from concourse.bass import ts, ds
    from concourse.masks import make_identity

    bf16 = mybir.dt.bfloat16
    f32 = mybir.dt.float32
    Exp = mybir.ActivationFunctionType.Exp

    nc = bacc.Bacc(trn_type="TRN2", target_bir_lowering=False, debug=False)

    xt = nc.dram_tensor("xt", [HID, T], bf16, kind="ExternalInput").ap()
    # weights host-packed [p, a*128+c] = W[a*128+p, c]: 2 KB contiguous per
    # partition so the DMA runs at full descriptor width
    wq = nc.dram_tensor("wq", [P, HID], bf16, kind="ExternalInput").ap()
    wk = nc.dram_tensor("wk", [P, HID], bf16, kind="ExternalInput").ap()
    wv = nc.dram_tensor("wv", [P, HID], bf16, kind="ExternalInput").ap()
    bias = nc.dram_tensor("bias", [P, 2], f32, kind="ExternalInput").ap()
    out = nc.dram_tensor("out", [T, P], f32, kind="ExternalOutput").ap()

    with tile.TileContext(nc) as tc:
        with (
            tc.tile_pool(name="const", bufs=1) as cpool,
            tc.tile_pool(name="xtp", bufs=1) as xtpool,
            tc.tile_pool(name="qkv", bufs=1) as qkvpool,
            tc.tile_pool(name="pt", bufs=1) as ptpool,
            tc.tile_pool(name="small", bufs=4) as smallpool,
            tc.tile_pool(name="ot", bufs=2) as otpool,
            tc.tile_pool(name="ps", bufs=2, space="PSUM") as psp,
        ):
            # X^T half-buffer: holds one batch's tokens; batch 1 reloads it
            # (all batch-0 projections are emitted before the reload DMAs).
            xt_sb = xtpool.tile([P, HK, S], bf16, tag="xt")
            xtp = xt.rearrange("(a p) t -> p a t", p=P)
            bias_sb = cpool.tile([P, 2], f32, tag="bias")
            b_sbs = [bias_sb[:, i : i + 1] for i in range(2)]
            wq_sb = cpool.tile([P, HK, P], bf16, tag="wq")
            wk_sb = cpool.tile([P, HK, P], bf16, tag="wk")
            wv_sb = cpool.tile([P, HK, P], bf16, tag="wv")
            w_sbs = [wq_sb, wk_sb]
            # DMA arrival order matched to first-consumption order; transfers
            # serialize on the (single-slot) DMA fabric in queue order, so the
            # first xt quarter is split in two 256-token pieces with the
            # (small) weight transfers slotted between them.
            nc.sync.dma_start(wq_sb, wq.rearrange("p (a c) -> p a c", a=HK))
            nc.sync.dma_start(xt_sb[:, :, 0:256], xtp[:, :, 0:256])
            nc.sync.dma_start(wk_sb, wk.rearrange("p (a c) -> p a c", a=HK))
            nc.sync.dma_start(bias_sb, bias)
            nc.sync.dma_start(xt_sb[:, :, 256:512], xtp[:, :, 256:512])
            nc.sync.dma_start(xt_sb[:, :, ts(1, 512)], xtp[:, :, ts(1, 512)])
            nc.sync.dma_start(wv_sb, wv.rearrange("p (a c) -> p a c", a=HK))
            for quarter in range(2, 4):
                nc.sync.dma_start(
                    xt_sb[:, :, ts(quarter, 512)], xtp[:, :, ts(quarter, 512)]
                )

            ident_bf = cpool.tile([P, P], bf16, tag="identb")
            make_identity(nc, ident_bf)

            qt_sb = qkvpool.tile([P, T], bf16, tag="qt")
            kt_sb = qkvpool.tile([P, T], bf16, tag="kt")
            # V' per head: [k-part, global ktile, 65]; col 64 = ones (row sums)
            vp_sb = qkvpool.tile([P, 2, T // P, D + 1], bf16, tag="vp")
            nc.vector.memset(vp_sb[:, :, :, D : D + 1], 1.0)

            # PE warm-up while the first DMAs land: identity-only matmuls
            # ramp the HAM clock gate to full speed before real work. The
            # accumulated result is read once (into a V' slot that a later
            # v_proj overwrites) so DCE keeps the chain.
            NWU = 38
            wu = psp.tile([P, P], f32, tag="pj", bufs=2, name="wups")
            for i in range(NWU):
                nc.tensor.matmul(
                    wu, ident_bf, ident_bf, start=(i == 0), stop=(i == NWU - 1)
                )
            nc.vector.tensor_copy(vp_sb[:, 0, 0, 0:D], wu[:, 0:D])

            def qk_proj(t8, which, half=None):
                """Project 512 tokens (chunk t8) for q/k (which=0/1).

                half=0/1 projects only 256 tokens (startup granularity)."""
                w_sb, b_sb = w_sbs[which], b_sbs[which]
                dst = (qt_sb, kt_sb)[which]
                if half is None:
                    t0, w = 0, 512
                elif isinstance(half, tuple):
                    t0, w = half
                else:
                    t0, w = half * 256, 256
                ps = psp.tile([P, 512], f32, tag="pj", bufs=2, name="projps")
                ps = ps[:, 0:w]
                for a in range(HK):
                    nc.tensor.matmul(
                        ps,
                        w_sb[:, a, :],
                        xt_sb[:, a, ds((t8 % 4) * 512 + t0, w)],
                        start=(a == 0),
                        stop=(a == HK - 1),
                    )
                # the bias-add gates later score groups: let it beat the
                # deferred fillers on DVE
                with tc.high_priority():
                    nc.vector.tensor_scalar_add(
                        dst[:, ds(t8 * 512 + t0, w)], ps, b_sb
                    )

            def v_proj_tile(t8, tt4):
                """Project one 128-token tile of V directly into V'[k, d]."""
                psv = psp.tile([P, P], f32, tag="pj", bufs=2, name="vps")
                for a in range(HK):
                    nc.tensor.matmul(
                        psv,
                        xt_sb[:, a, ds((t8 % 4) * 512 + tt4 * P, P)],
                        wv_sb[:, a, :],
                        start=(a == 0),
                        stop=(a == HK - 1),
                    )
                gk = (t8 // 4) * 16 + (t8 % 4) * 4 + tt4
                for h in range(2):
                    nc.vector.tensor_copy(
                        vp_sb[:, h, gk, 0:D], psv[:, h * D : (h + 1) * D]
                    )

            # PT ring: 2 heads x 32 slots x [128, 1024] bf16 (128 KB/partition)
            RING = 32
            pt_all = ptpool.tile([P, 2, RING, 1024], bf16, tag="pt")

            ot_cur = {}

            def pv_norm(unit, head, qt, ctx):
                """Reciprocal-normalize one finished ctx tile + chunked DMA."""
                b = unit // 2
                rc = smallpool.tile([P, 1], f32, tag="rc")
                nc.vector.reciprocal(rc, ctx[:, D : D + 1])
                if qt == 0:
                    ot_cur[(unit, head)] = otpool.tile(
                        [P, 8, D], f32, tag="ot", name="ot"
                    )
                ot = ot_cur[(unit, head)]
                nc.vector.tensor_scalar_mul(ot[:, qt, :], ctx[:, 0:D], rc)
                # chunked DMAs so earlier chunks overlap later P@V; the very
                # last (unit, head) uses finer chunks to shrink the tail
                chunks = {3: 4, 5: 2, 7: 2} if (unit, head) == (3, 1) else {3: 4, 7: 4}
                if qt in chunks:
                    nq = chunks[qt]
                    q0 = qt + 1 - nq
                    qbase = b * S + (unit % 2) * 1024 + q0 * 128
                    hb = D * head
                    dst = out[ds(qbase, nq * P), ds(hb, D)].rearrange(
                        "(tt p) d -> p tt d", p=P
                    )
                    nc.sync.dma_start(dst, ot[:, q0 : qt + 1, :])

            def pv_tile(unit, head, qt, tag="ctx"):
                """P@V + normalize for one (unit, head, 128-token q tile).

                ctx[q, d|sum] accumulates over the 16 buffered PT k-tiles
                with PT as the stationary operand (65 moving cols), then a
                per-partition reciprocal-normalize; no transpose needed."""
                b = unit // 2
                ctx = psp.tile([P, D + 1], f32, tag=tag, bufs=2, name="ctx")
                for kt in range(16):
                    nc.tensor.matmul(
                        ctx,
                        pt_all[:, head, (unit * 16 + kt) % RING, ds(qt * P, P)],
                        vp_sb[:, head, b * 16 + kt, :],
                        start=(kt == 0),
                        stop=(kt == 15),
                    )
                pv_norm(unit, head, qt, ctx)

            # Tail pre-accumulation for the last (unit 3, head 1) P@V: q
            # tiles 0-3 accumulate kt 0..14 under the last exps' shadow on
            # the 4 free PSUM banks (ctx + the by-then-idle pj tag), leaving
            # only the kt15 matmul + normalize after the final exp.
            pv31_ctx = {}

            def pv31_open(qt, tag):
                ctx = psp.tile([P, D + 1], f32, tag=tag, bufs=2, name="ctx31")
                for kt in range(15):
                    nc.tensor.matmul(
                        ctx,
                        pt_all[:, 1, (48 + kt) % RING, ds(qt * P, P)],
                        vp_sb[:, 1, 16 + kt, :],
                        start=(kt == 0),
                        stop=False,
                    )
                pv31_ctx[qt] = ctx

            def pv31_close(qt):
                ctx = pv31_ctx[qt]
                nc.tensor.matmul(
                    ctx,
                    pt_all[:, 1, 31, ds(qt * P, P)],
                    vp_sb[:, 1, 31, :],
                    start=False,
                    stop=True,
                )
                pv_norm(3, 1, qt, ctx)

            # Deferred-work queue: (cost, fn, deadline). Deadline (u, kt)
            # means the item MUST be emitted before (u, kt)'s scores/exp —
            # emission order is Tile's semantic order, so a late RAW
            # producer or a PT-ring WAR reader would read wrong data.
            # Items are popped by deadline (forced) or by cost pacing.
            # Unit 3 is head-phased: its kt key runs 0..31 (head*16 + kt).
            work_q = deque()

            def q_proj(t8, which, dl):
                work_q.append((1.7, lambda: qk_proj(t8, which), dl))

            def q_vproj(t8, dl):
                for tt4 in range(4):
                    work_q.append(
                        (0.5, lambda t=tt4: v_proj_tile(t8, t), dl)
                    )

            def q_pv(unit, dl, heads=(0, 1)):
                for head in heads:
                    for qt in range(8):
                        work_q.append(
                            (
                                0.5,
                                lambda h=head, q=qt: pv_tile(unit, h, q),
                                dl,
                            )
                        )

            NEVER = (9, 0)

            def push_unit_work(unit):
                # Projections first (they gate later score groups and so the
                # ACT-critical chain); the scheduler runs everything by
                # readiness with emission order as the tiebreak, so pv/v
                # fillers naturally yield to them.
                if unit == 0:
                    # rest of batch 0 (essentials q0,q1,k0 already emitted)
                    q_proj(1, 1, (0, 4))  # k1
                    q_proj(2, 1, (0, 8))  # k2
                    q_proj(3, 1, (0, 12))  # k3
                    q_proj(2, 0, (1, 0))  # q2 (unit 1 scores)
                    q_proj(3, 0, (1, 0))  # q3
                    q_vproj(0, (1, 0))  # v0..v3 feed pv(0) in unit 1
                    q_vproj(1, (1, 0))
                    q_vproj(2, (1, 0))
                    q_vproj(3, (1, 0))
                elif unit == 1:
                    q_proj(4, 1, (2, 0))  # k4
                    q_proj(4, 0, (2, 0))  # q4
                    q_proj(5, 0, (2, 0))  # q5
                    q_pv(0, (2, 0))  # PT slots reused by unit 2
                    q_vproj(4, (3, 0))  # v4 feeds pv(2) in unit 3
                elif unit == 2:
                    q_proj(5, 1, (2, 4))  # k5
                    q_proj(6, 1, (2, 8))  # k6
                    q_proj(7, 1, (2, 12))  # k7
                    q_proj(6, 0, (3, 0))  # q6
                    q_proj(7, 0, (3, 0))  # q7
                    q_pv(1, (3, 0))  # PT slots reused by unit 3
                    q_vproj(5, (3, 0))
                elif unit == 3:
                    # rest of batch 1's V' (feeds pv(2); FIFO keeps them
                    # ahead), then pv(2) under phase-A's exp shadow
                    q_vproj(6, NEVER)
                    q_vproj(7, NEVER)
                    q_pv(2, NEVER)

            def do_scores_exp(unit, head, kt, key):
                """One (head, kt): 2 score matmuls + 1 exp, plus queue pops.

                High priority: the score->exp chain is the ACT critical path,
                so score matmuls must preempt deferred fillers on the PE the
                moment their st WAR clears."""
                b, qh = unit // 2, unit % 2
                base = b * S
                qbase = base + qh * 1024
                st = psp.tile([P, 1024], f32, tag="st", bufs=2, name="st")
                hb = D * head
                with tc.high_priority():
                    for j in range(2):
                        nc.tensor.matmul(
                            st[:, ts(j, 512)],
                            kt_sb[ds(hb, D), ds(base + kt * P, P)],
                            qt_sb[ds(hb, D), ds(qbase + j * 512, 512)],
                            start=True,
                            stop=True,
                        )
                    nc.scalar.activation(
                        pt_all[:, head, (unit * 16 + kt) % RING, :],
                        st,
                        Exp,
                        scale=0.125,
                    )

            def scores_exp_part_u3h1(kt, j):
                """One 512-wide j-half of unit 3 / head 1's scores+exp."""
                hb = D
                slot = (48 + kt) % RING
                stj = psp.tile([P, 512], f32, tag="st", bufs=2, name="stj3")
                with tc.high_priority():
                    nc.tensor.matmul(
                        stj,
                        kt_sb[ds(hb, D), ds(S + kt * P, P)],
                        qt_sb[ds(hb, D), ds(S + 1024 + j * 512, 512)],
                        start=True,
                        stop=True,
                    )
                    nc.scalar.activation(
                        pt_all[:, 1, slot, ds(j * 512, 512)],
                        stj,
                        Exp,
                        scale=0.125,
                    )

            def scores_exp_part(head, kt, c0, w):
                """Unit-0 startup: one w-wide score matmul + exp so ACT can
                start before the full q half (and later k tiles) arrive.
                h0 uses the st banks, h1 the (startup-idle) ctx banks so the
                four in-flight tiles never WAR-serialize."""
                hb = D * head
                stj = psp.tile(
                    [P, 512], f32, tag=("st", "ctx")[head], bufs=2, name="stj"
                )
                with tc.high_priority():
                    nc.tensor.matmul(
                        stj[:, 0:w],
                        kt_sb[ds(hb, D), ds(kt * P, P)],
                        qt_sb[ds(hb, D), ds(c0, w)],
                        start=True,
                        stop=True,
                    )
                    nc.scalar.activation(
                        pt_all[:, head, kt, ds(c0, w)],
                        stj[:, 0:w],
                        Exp,
                        scale=0.125,
                    )

            # ---- batch 0 startup: 256-token projection granularity and
            # split scores/exps for kt 0..3 so the first exp runs as soon
            # as the first 256 tokens + wq/wk have landed ----
            qk_proj(0, 0, half=0)  # q0a
            qk_proj(0, 1, half=0)  # k0a -> ktiles 0,1
            for kt in (0, 1):
                for head in range(2):
                    scores_exp_part(head, kt, 0, 256)
            qk_proj(0, 0, half=1)  # q0b
            for kt in (0, 1):
                for head in range(2):
                    scores_exp_part(head, kt, 256, 256)
            qk_proj(0, 1, half=1)  # k0b -> ktiles 2,3
            for kt in (2, 3):
                for head in range(2):
                    scores_exp_part(head, kt, 0, 512)
            qk_proj(1, 0)  # q1
            for kt in range(4):
                for head in range(2):
                    scores_exp_part(head, kt, 512, 512)

            credit = 2.0
            for unit in range(4):
                if unit == 1:
                    # drain every batch-0 consumer of xt_sb first: emission
                    # order is semantic order, so the reload must be emitted
                    # after all batch-0 projection reads
                    while work_q and work_q[0][2] <= (1, 0):
                        work_q.popleft()[1]()
                    # reload X^T with batch 1 tokens (WAR on batch-0 projs)
                    for quarter in range(4):
                        nc.sync.dma_start(
                            xt_sb[:, :, ts(quarter, 512)],
                            xtp[:, :, ds(S + quarter * 512, 512)],
                        )
                push_unit_work(unit)
                if unit == 0:
                    steps = [(kt, (0, 1)) for kt in range(4, 16)]
                elif unit < 3:
                    steps = [(kt, (0, 1)) for kt in range(16)]
                else:
                    # head-phased: h0's 16 kt, then h1's (kt key 0..31)
                    steps = [(kt, (kt // 16,)) for kt in range(32)]
                for kkey, heads in steps:
                    while work_q and work_q[0][2] <= (unit, kkey):
                        _, fn, _ = work_q.popleft()
                        fn()
                    if unit == 3 and kkey == 31:
                        # j-split the very last exp: the tail P@V for q tiles
                        # 0-3 only needs the j0 half, so it closes right
                        # after it while the j1 half still runs
                        scores_exp_part_u3h1(15, 0)
                        for qt in range(4):
                            pv31_close(qt)
                        scores_exp_part_u3h1(15, 1)
                        continue
                    for head in heads:
                        do_scores_exp(unit, head, kkey % 16, kkey)
                    if unit == 3 and kkey == 30:
                        # kt 0..14 PT tiles are final: pre-accumulate the
                        # tail's first 4 q tiles under the last exps
                        for qt, tag in ((0, "ctx"), (1, "ctx"), (2, "pj"), (3, "pj")):
                            pv31_open(qt, tag)
                    if unit == 3 and kkey == 15:
                        # phase B begins: h0's PT tiles are final, its P@V
                        # runs under h1's exp shadow
                        q_pv(3, NEVER, heads=(0,))
                    # deferred work drained under the exp shadow, paced so
                    # the PE never runs far ahead of ACT
                    credit = min(credit + (1.4 if unit < 3 else 0.7), 8.0)
                    while work_q and work_q[0][0] <= credit:
                        cost, fn, _ = work_q.popleft()
                        credit -= cost
                        fn()
            while work_q:
                work_q.popleft()[1]()
            # tail: only h1's last 4 q tiles remain (0-3 closed in-loop).
            # qt4/5 take the st banks (free the moment the last exps read
            # them); qt6/7 take the slots qt0/qt2's norms release.
            for qt, tag in ((4, "st"), (5, "st"), (6, "ctx"), (7, "pj")):
                pv_tile(3, 1, qt, tag=tag)

    nc.compile()
    return nc


def get_nc():
    if "nc" not in _CACHED:
        _CACHED["nc"] = _build()
    return _CACHED["nc"]


def kernel(hidden_states, Wq, bq, Wk, bk, Wv, bv):
    from concourse.bass_utils import run_bass_kernel_spmd

    nc = get_nc()

    x2 = np.asarray(hidden_states, dtype=np.float32).reshape(T, HID)
    xt_b = np.ascontiguousarray(x2.T).astype(BF)
    bv_f = np.asarray(bv, np.float32)

    def pack_w(W, sl):
        # [p, a*128+c] = W[a*128+p, c]: 2 KB contiguous rows for fast DMA
        w = np.asarray(W, np.float32)[:, sl].reshape(HK, P, P)
        return np.ascontiguousarray(w.transpose(1, 0, 2).reshape(P, HID)).astype(BF)

    in_maps = []
    for c in range(N_CORES):
        sl = slice(P * c, P * (c + 1))
        in_maps.append(
            {
                "xt": xt_b,
                "wq": pack_w(Wq, sl),
                "wk": pack_w(Wk, sl),
                "wv": pack_w(Wv, sl),
                "bias": np.ascontiguousarray(
                    np.stack(
                        [
                            np.asarray(bq, np.float32)[sl],
                            np.asarray(bk, np.float32)[sl],
                        ],
                        axis=1,
                    )
                ),
            }
        )

    res = run_bass_kernel_spmd(nc, in_maps, list(range(N_CORES)))

    full = np.empty((T, HID), dtype=np.float32)
    for c in range(N_CORES):
        # V bias: softmax rows sum to 1, so ctx(V + bv) = ctx(V) + bv exactly
        full[:, P * c : P * (c + 1)] = res.results[c]["out"] + bv_f[P * c : P * (c + 1)]
    return full.reshape(B, S, HID)
